# revision 7
# baseline (speedup 1.0000x reference)
"""nn_Net_43860206026847: GRU-like net on 8 trn2 NeuronCores (Bass/Tile).

Strategy (v3)
-------------
Data-parallel over batch (8 rows/core, params replicated), single fused
pass per core:

  * Markov projection folded into the gates (Wp_g = Wg[:, :H] @ Wm); the
    scan keeps only the h-dependent halves A_g = Wg[:, H:].
  * Projections Ug run on-the-fly per 32-step block in bf16 straight into
    SBUF (no DRAM round trip), pre-scaled by WS so they share the fp8
    matmul scale; the descale rides the activation `scale` input.
  * Scan matmuls: fp8(e4m3) weights+operands, plain 128x128 stationary
    tiles (halves the PE weight-load bytes vs fp16).  The scan is
    weight-load bound: 3*H*H weights/step stream through the PE load port.
  * Ug[t] is copied into PSUM before the step's matmuls; all scan matmuls
    accumulate with start=False on top, so sigmoid/tanh read PSUM
    directly - no separate add, and no start-bit zero-region hazards.
  * Elementwise chain split in feature halves; projection groups for the
    next block are interleaved between scan steps as PE filler.
"""

import numpy as np
import ml_dtypes
from contextlib import ExitStack

import concourse.bass as bass
import concourse.tile as tile
from concourse import bacc, mybir
from concourse import bass_utils

B, S, D, H = 64, 512, 768, 1024
NCORES = 8
BL = B // NCORES      # 8 batch rows per core
P = 128
DC = D // P           # 6 contraction chunks over D
HC = H // P           # 8 chunks over H
TB = 32               # scan time-block (projection granularity)
NBLK = S // TB
WS = 1024.0           # scan weight scale (max |A_g|*WS ~ 118 < 240)
INV = 1.0 / WS

F32 = mybir.dt.float32
BF16 = mybir.dt.bfloat16

SCAN_FP8 = False
if SCAN_FP8:
    SDT = mybir.dt.float8e4
    SNP = ml_dtypes.float8_e4m3
else:
    SDT = mybir.dt.float16
    SNP = np.float16


def _host_prep(x, Wm, bm, Wh, bh, Wz, bz, Wr, br, Wi, bi):
    f8 = np.float64
    Wg = [np.asarray(w) for w in (Wz, Wr, Wi)]
    bg = [np.asarray(b) for b in (bz, br, bi)]
    Wp = [np.asarray(W, f8)[:, :H] @ np.asarray(Wm, f8) for W in Wg]
    bp = [np.asarray(b, f8) + np.asarray(W, f8)[:, :H] @ np.asarray(bm, f8)
          for W, b in zip(Wg, bg)]

    WprojT = np.empty((3, DC, P, H), ml_dtypes.bfloat16)
    for g in range(3):
        WprojT[g] = Wp[g].T.astype(np.float32).reshape(DC, P, H)
    bprj = np.stack([(b * WS).astype(np.float32).reshape(HC, P) for b in bp])

    # scan weights: [g, p, kc, jc, m] so (kc, jc) tiles are contiguous
    Ws8 = np.empty((3, P, HC, HC, P), SNP)
    for g in range(3):
        A8 = (np.asarray(Wg[g], np.float32)[:, H:] * WS).astype(SNP)
        Ws8[g] = A8.reshape(HC, P, HC, P).transpose(3, 2, 0, 1)
    Ws8 = np.ascontiguousarray(Ws8.reshape(3, P, HC * HC * P))

    WhT = np.ascontiguousarray(np.asarray(Wh, np.float32).T).reshape(DC, P, H)
    bh_r = np.asarray(bh, np.float32).reshape(HC, P)

    x = np.asarray(x, np.float32)
    in_maps = []
    for c in range(NCORES):
        xc = x[c * BL:(c + 1) * BL]
        xT = np.ascontiguousarray(
            xc.transpose(2, 1, 0).reshape(DC, P, S * BL).astype(ml_dtypes.bfloat16))
        x0T = np.ascontiguousarray(xc[:, 0, :].T.reshape(DC, P, BL))
        in_maps.append({
            "xT": xT, "x0T": x0T, "WprojT": WprojT, "Ws8": Ws8,
            "WhT": WhT, "bprj": bprj, "bh": bh_r,
        })
    return in_maps


def _build_nc():
    nc = bacc.Bacc("TRN2", target_bir_lowering=False, debug=False,
                   num_devices=NCORES)

    xT_in = nc.dram_tensor("xT", [DC, P, S * BL], BF16, kind="ExternalInput").ap()
    x0T_in = nc.dram_tensor("x0T", [DC, P, BL], F32, kind="ExternalInput").ap()
    wproj_in = nc.dram_tensor("WprojT", [3, DC, P, H], BF16, kind="ExternalInput").ap()
    ws8_in = nc.dram_tensor("Ws8", [3, P, HC * HC * P], SDT, kind="ExternalInput").ap()
    wh_in = nc.dram_tensor("WhT", [DC, P, H], F32, kind="ExternalInput").ap()
    bprj_in = nc.dram_tensor("bprj", [3, HC, P], F32, kind="ExternalInput").ap()
    bh_in = nc.dram_tensor("bh", [HC, P], F32, kind="ExternalInput").ap()
    hout = nc.dram_tensor("hout", [HC, P, BL], F32, kind="ExternalOutput").ap()

    sig = mybir.ActivationFunctionType.Sigmoid
    tanh = mybir.ActivationFunctionType.Tanh
    ident = mybir.ActivationFunctionType.Identity
    copyf = mybir.ActivationFunctionType.Copy
    SUB = mybir.AluOpType.subtract
    MUL = mybir.AluOpType.mult
    ADD = mybir.AluOpType.add

    with tile.TileContext(nc) as tc, ExitStack() as ctx:
        pers = ctx.enter_context(tc.tile_pool(name="pers", bufs=1))

        wproj_sb = pers.tile([P, 3 * DC * H], BF16)
        for g in range(3):
            for kc in range(DC):
                nc.sync.dma_start(
                    wproj_sb[:, (g * DC + kc) * H:(g * DC + kc + 1) * H],
                    wproj_in[g, kc])
        WSZ = HC * HC * P
        ws8_sb = pers.tile([P, 3 * WSZ], SDT)
        for g in range(3):
            nc.sync.dma_start(ws8_sb[:, g * WSZ:(g + 1) * WSZ], ws8_in[g])
        bprj_sb = pers.tile([P, 3 * HC], F32)
        for g in range(3):
            nc.sync.dma_start(bprj_sb[:, g * HC:(g + 1) * HC],
                              bprj_in[g].rearrange("h p -> p h"))
        bh_sb = pers.tile([P, HC], F32)
        nc.sync.dma_start(bh_sb[:], bh_in.rearrange("h p -> p h"))

        def ws8_tile(g, kc, jc):
            base = g * WSZ + (kc * HC + jc) * P
            return ws8_sb[:, base:base + P]

        hpool = ctx.enter_context(tc.tile_pool(name="hpool", bufs=2))
        tmppool = ctx.enter_context(tc.tile_pool(name="tmppool", bufs=2))
        ugzr_pool = ctx.enter_context(tc.tile_pool(name="ugzr", bufs=2))
        ugi_pool = ctx.enter_context(tc.tile_pool(name="ugi", bufs=2))
        xblk_pool = ctx.enter_context(tc.tile_pool(name="xblk", bufs=2))
        psA = ctx.enter_context(tc.tile_pool(name="psA", bufs=2, space="PSUM"))
        pszr = ctx.enter_context(tc.tile_pool(name="pszr", bufs=2, space="PSUM"))
        psi = ctx.enter_context(tc.tile_pool(name="psi", bufs=2, space="PSUM"))

        # ---------------- h0 = x0 @ Wh.T + bh ----------------
        h_f32 = hpool.tile([P, HC * BL], F32, tag="h")
        h8 = hpool.tile([P, HC * BL], SDT, tag="h8")
        with ExitStack() as bctx:
            bpool = bctx.enter_context(tc.tile_pool(name="bpool", bufs=1))
            whT_sb = bpool.tile([P, DC * H], F32)
            for kc in range(DC):
                nc.sync.dma_start(whT_sb[:, kc * H:(kc + 1) * H], wh_in[kc])
            x0t = bpool.tile([P, DC * BL], F32)
            for kc in range(DC):
                nc.sync.dma_start(x0t[:, kc * BL:(kc + 1) * BL], x0T_in[kc])
            for fc in range(HC):
                psB = psi.tile([P, HC * BL], F32, tag="i")
                for kc in range(DC):
                    nc.tensor.matmul(
                        psB[:, :BL],
                        whT_sb[:, kc * H + fc * P: kc * H + (fc + 1) * P],
                        x0t[:, kc * BL:(kc + 1) * BL],
                        start=(kc == 0), stop=(kc == DC - 1))
                nc.any.tensor_scalar_add(h_f32[:, fc * BL:(fc + 1) * BL],
                                         psB[:, :BL], bh_sb[:, fc:fc + 1])
            nc.vector.tensor_copy(h8[:], h_f32[:])

        # ---------------- fused projection + scan ----------------
        def make_proj(blk):
            xt = xblk_pool.tile([P, DC * TB * BL], BF16, tag="xt")
            for kc in range(DC):
                nc.sync.dma_start(
                    xt[:, kc * TB * BL:(kc + 1) * TB * BL],
                    xT_in[kc, :, blk * TB * BL:(blk + 1) * TB * BL])
            ug_zr = ugzr_pool.tile([P, TB * 2 * HC * BL], F32, tag="ugzr")
            ug_i = ugi_pool.tile([P, TB * HC * BL], F32, tag="ugi")
            zr_v = ug_zr[:].rearrange("p (t g j b) -> p t g j b", t=TB, g=2, j=HC)
            i_v = ug_i[:].rearrange("p (t j b) -> p t j b", t=TB, j=HC)

            def group(g, fc):
                def emit():
                    pA = psA.tile([P, TB * BL], F32, tag="pA")
                    for kc in range(DC):
                        nc.tensor.matmul(
                            pA[:],
                            wproj_sb[:, (g * DC + kc) * H + fc * P:
                                     (g * DC + kc) * H + (fc + 1) * P],
                            xt[:, kc * TB * BL:(kc + 1) * TB * BL],
                            start=(kc == 0), stop=(kc == DC - 1))
                    dst = zr_v[:, :, g, fc, :] if g < 2 else i_v[:, :, fc, :]
                    nc.scalar.activation(dst, pA[:], ident,
                                         bias=bprj_sb[:, g * HC + fc:g * HC + fc + 1],
                                         scale=WS)
                return emit
            groups = [group(g, fc) for g in range(3) for fc in range(HC)]
            return ug_zr, ug_i, groups

        def emit_scan_block(ug_zr, ug_i, filler):
            nonlocal h_f32, h8
            ugzr_f = ug_zr[:]
            ugi_f = ug_i[:]

            def preload(tau):
                # Ug[tau] into PSUM; the step's matmuls accumulate on top.
                # Issued one step early so it clears the DVE queue before
                # the PE needs the bank.
                ps_zr = pszr.tile([P, 2 * HC * BL], F32, tag="zr")
                ps_i = psi.tile([P, HC * BL], F32, tag="i")
                nc.vector.tensor_copy(
                    ps_zr[:], ugzr_f[:, tau * 2 * HC * BL:(tau + 1) * 2 * HC * BL])
                nc.vector.tensor_copy(
                    ps_i[:], ugi_f[:, tau * HC * BL:(tau + 1) * HC * BL])
                return ps_zr, ps_i

            ps_next = preload(0)
            for tau in range(TB):
                h_prev, h8_prev = h_f32, h8
                hv = h_prev[:].rearrange("p (j b) -> p j b", j=HC)
                h8v = h8_prev[:].rearrange("p (k b) -> p k b", k=HC)

                ps_zr, ps_i = ps_next
                zrp = ps_zr[:].rearrange("p (g j b) -> p g j b", g=2, j=HC)
                zr_sb = tmppool.tile([P, 2 * HC * BL], F32, tag="zrsb")
                zrv = zr_sb[:].rearrange("p (g j b) -> p g j b", g=2, j=HC)
                rh8 = tmppool.tile([P, HC * BL], SDT, tag="rh8")
                rh8v = rh8[:].rearrange("p (k b) -> p k b", k=HC)
                piv = ps_i[:].rearrange("p (j b) -> p j b", j=HC)
                hp = tmppool.tile([P, HC * BL], F32, tag="hp")
                d = tmppool.tile([P, HC * BL], F32, tag="d")
                zd = tmppool.tile([P, HC * BL], F32, tag="zd")
                h_new = hpool.tile([P, HC * BL], F32, tag="h")
                h8_new = hpool.tile([P, HC * BL], SDT, tag="h8")

                def zr_mm(g, jc, kcs):
                    for kc in kcs:
                        nc.tensor.matmul(
                            zrp[:, g, jc, :], ws8_tile(g, kc, jc), h8v[:, kc],
                            start=False, stop=(kc == HC - 1),
                            skip_group_check=True)

                def i_mm(jc, kcs):
                    for kc in kcs:
                        nc.tensor.matmul(
                            piv[:, jc, :], ws8_tile(2, kc, jc), rh8v[:, kc],
                            start=False, stop=(kc == HC - 1),
                            skip_group_check=True)

                KH0, KH1 = range(0, 4), range(4, HC)
                # z,r: lower contraction half first (only needs h8 half 0)
                for g in range(2):
                    for jc in range(HC):
                        zr_mm(g, jc, KH0)
                for g in range(2):
                    for jc in range(4):
                        zr_mm(g, jc, KH1)
                # sigma + r*h for feature half 0
                nc.scalar.activation(zrv[:, :, 0:4, :], zrp[:, :, 0:4, :],
                                     sig, scale=INV)
                nc.vector.tensor_tensor(rh8[:, 0:4 * BL], zrv[:, 1, 0:4, :],
                                        hv[:, 0:4, :], MUL)
                for g in range(2):
                    for jc in range(4, HC):
                        zr_mm(g, jc, KH1)
                nc.scalar.activation(zrv[:, :, 4:, :], zrp[:, :, 4:, :],
                                     sig, scale=INV)
                nc.vector.tensor_tensor(rh8[:, 4 * BL:], zrv[:, 1, 4:, :],
                                        hv[:, 4:, :], MUL)

                if tau + 1 < TB:
                    ps_next = preload(tau + 1)

                # candidate gate
                for jc in range(HC):
                    i_mm(jc, KH0)
                for jc in range(4):
                    i_mm(jc, KH1)
                for jh in range(2):
                    jsl = slice(jh * 4, jh * 4 + 4)
                    csl = slice(jh * 4 * BL, (jh * 4 + 4) * BL)
                    if jh == 1:
                        for jc in range(4, HC):
                            i_mm(jc, KH1)
                    nc.scalar.activation(hp[:, csl], piv[:, jsl, :],
                                         tanh, scale=INV)
                    nc.vector.tensor_tensor(d[:, csl], hp[:, csl],
                                            h_prev[:, csl], SUB)
                    nc.vector.tensor_tensor(
                        zd[:, csl], zrv[:, 0, jsl, :],
                        d[:].rearrange("p (j b) -> p j b", j=HC)[:, jsl, :], MUL)
                    nc.vector.tensor_tensor(h_new[:, csl], h_prev[:, csl],
                                            zd[:, csl], ADD)
                    nc.vector.tensor_copy(h8_new[:, csl], h_new[:, csl])

                if filler:
                    filler.pop(0)()

                h_f32, h8 = h_new, h8_new

        prev = None
        for blk in range(NBLK):
            ug_zr, ug_i, groups = make_proj(blk)
            if prev is None:
                for e in groups:
                    e()
                prev = (ug_zr, ug_i)
            else:
                emit_scan_block(*prev, filler=groups)
                prev = (ug_zr, ug_i)
        emit_scan_block(*prev, filler=[])

        for fc in range(HC):
            nc.sync.dma_start(hout[fc], h_f32[:, fc * BL:(fc + 1) * BL])

    nc.compile()
    return nc


_NC_CACHE = None


def kernel(**inputs) -> np.ndarray:
    global _NC_CACHE
    in_maps = _host_prep(**{k: np.asarray(v) for k, v in inputs.items()})
    if _NC_CACHE is None:
        _NC_CACHE = _build_nc()
    res = bass_utils.run_bass_kernel_spmd(
        _NC_CACHE, in_maps, core_ids=list(range(NCORES)), trace=False)
    out = np.empty((B, 1, H), np.float32)
    for c, r in enumerate(res.results):
        out[c * BL:(c + 1) * BL, 0, :] = r["hout"].transpose(2, 0, 1).reshape(BL, H)
    return out
